# revision 20
# baseline (speedup 1.0000x reference)
"""Trainium2 Bass kernel for nn_MultiHeadAttention_18700287607660.

Math (B=128, L=500, D=512, NWAY=5, n_head=1):
  qp = q@Wq.T ; kp = k@Wk.T ; vp = v@Wv.T
  attn_avg = softmax(mean_over_groups(qp @ kp.T / temp))     # [B, 5, L]
  proto = attn_avg @ vp                                      # [B, 5, D]
  out1 = LN1(broadcast(proto) + kp)
  out  = LN2(leaky_relu(out1@Wfc.T, 0.1) + out1)

Key restructurings (exact up to fp reassociation):
  * mean happens BEFORE softmax, so the [500,500] attention matrix is never
    formed:  S = (Sel @ q) @ (Wq.T @ Wk / temp) @ k.T   with Sel the [5,500]
    group-mean selector. Wqk = Wq.T@Wk/temp folded on the host.
  * proto = (A @ v) @ Wv.T  — the V projection is never materialized.
  * broadcast(proto) is a K=5 matmul accumulated straight into kp's PSUM.

Implementation choices:
  * matmul operands in fp16 — fp32 moving operands stream at half rate on the
    PE; fp16 keeps ~1e-3 accuracy with fp32 PSUM accumulation.
  * seq dim host-padded 500->512 (zero rows) so the DMA xbar transpose engine
    (2-byte dtypes, rows%16==0) produces k^T and x^T — no PE transposes or
    PSUM->SBUF copybacks for the two big transposed tensors.
  * LayerNorm rstd via exp(-0.5*ln(var+eps)): keeps every ACT function in the
    one "natural_log_exp_and_others" table set (one table load total).

Sharding: pure data parallel, 16 batches per core across 8 cores.
"""
import os
import sys

for _p in ("/opt/trn_rl_repo", "/root/.axon_site/_ro/trn_rl_repo"):
    if os.path.isdir(_p) and _p not in sys.path:
        sys.path.insert(0, _p)

import numpy as np

import concourse.bacc as bacc
import concourse.bass as bass
import concourse.tile as tile
from concourse import mybir
from concourse.bass_utils import run_bass_kernel_spmd

F16 = mybir.dt.float16
F32 = mybir.dt.float32
N_CORES = 8
B = 128
BPC = B // N_CORES   # 16 batches per core
L = 500              # true seq len
LP = 512             # padded seq len (DMA xbar transpose needs rows%16==0)
LT = 128             # l-tile
NLT = LP // LT       # 4
LTAIL = L - 3 * LT   # 116 valid rows in the last l-tile
D = 512
DT = 128
NDT = D // DT        # 4
W = 5                # NWAY shot groups
TEMP = float(np.sqrt(float(D)))
EPS = 1e-6
LEAK = 0.1

# All ACT functions used here (Exp, Ln, Relu, Copy, Identity) live in the
# "natural_log_exp_and_others" table set, but bacc's per-activation greedy
# set chooser still flips between sets (hundreds of ~2.7us ACT_TABLE_LOADs).
# Empty out every other set (keeping positions, since act_func_set_id is the
# positional index into act_info.json) so exactly one set is ever loaded.
_orig_get_activation_tables = bacc.get_activation_tables


def _pinned_activation_tables(module_arch):
    tables = _orig_get_activation_tables(module_arch)
    if "natural_log_exp_and_others" in tables:
        return {
            name: (fns if name == "natural_log_exp_and_others" else set())
            for name, fns in tables.items()
        }
    return tables


bacc.get_activation_tables = _pinned_activation_tables


def _emit(nc, tc, ext, apply_gb):
    """Software-pipelined emission: per iteration s we emit
    load(s+2) | attn(s+1) | kp+LN1+xT(s) | fc+LN2+store(s-1)
    so every stage's inputs were produced in a previous iteration and the
    per-engine streams always have ready work from an adjacent batch.
    """
    import contextlib
    ctx = contextlib.ExitStack()
    with ctx:
        const = ctx.enter_context(tc.tile_pool(name="const", bufs=1))
        pin = ctx.enter_context(tc.tile_pool(name="pin", bufs=3))
        pkt = ctx.enter_context(tc.tile_pool(name="pkt", bufs=4))
        px = ctx.enter_context(tc.tile_pool(name="px", bufs=3))
        pxt = ctx.enter_context(tc.tile_pool(name="pxt", bufs=3))
        pt = ctx.enter_context(tc.tile_pool(name="pt", bufs=3))
        pr = ctx.enter_context(tc.tile_pool(name="pr", bufs=3))
        po = ctx.enter_context(tc.tile_pool(name="po", bufs=3))
        tiny = ctx.enter_context(tc.tile_pool(name="tiny", bufs=3))
        ptiny = ctx.enter_context(tc.tile_pool(name="ptiny", bufs=3))
        ps_small = ctx.enter_context(tc.tile_pool(name="ps_small", bufs=3, space="PSUM"))
        ps_kp = ctx.enter_context(tc.tile_pool(name="ps_kp", bufs=3, space="PSUM"))
        ps_fc = ctx.enter_context(tc.tile_pool(name="ps_fc", bufs=2, space="PSUM"))

        # ---- constants ----
        wkT_sb = const.tile([DT, NDT, D], F16)
        wvT_sb = const.tile([DT, NDT, D], F16)
        wfcT_sb = const.tile([DT, NDT, D], F16)
        wqk_sb = const.tile([DT, NDT, D], F16)
        for w_sb, name in ((wkT_sb, "wkT"), (wvT_sb, "wvT"),
                           (wfcT_sb, "wfcT"), (wqk_sb, "wqk")):
            nc.sync.dma_start(out=w_sb, in_=ext[name].rearrange("(i p) e -> p i e", p=DT))
        selT_sb = const.tile([LT, NLT, W], F16)
        nc.sync.dma_start(out=selT_sb, in_=ext["selT"].rearrange("(i p) w -> p i w", p=LT))
        bc5_sb = const.tile([W, NLT, LT], F16)
        nc.sync.dma_start(out=bc5_sb, in_=ext["bc5"].rearrange("w (i p) -> w i p", p=LT))
        id_sb = const.tile([W, W], F16)
        nc.sync.dma_start(out=id_sb, in_=ext["ident"][:])
        eps_sb = const.tile([DT, 1], F32)
        nc.vector.memset(eps_sb, EPS)
        gb_sb = {}
        if apply_gb:
            for name in ("g1", "b1", "g2", "b2"):
                t = const.tile([LT, D], F32)
                src = ext[name]
                bcast = bass.AP(tensor=src.tensor, offset=src.offset,
                                ap=[[0, LT]] + list(src.ap))
                nc.sync.dma_start(out=t, in_=bcast)
                gb_sb[name] = t

        state = {}

        def stage_load(b):
            st = state.setdefault(b, {})
            qv = pin.tile([LT, 2, NLT, D], F16, tag="qv", name=f"qv{b}")
            st["q"] = qv[:, 0, :, :]
            st["v"] = qv[:, 1, :, :]
            st["kT"] = pkt.tile([DT, NDT, LP], F16, tag="kT", name=f"kT{b}")
            nc.sync.dma_start(out=qv, in_=ext["qv"][b].rearrange("t (i p) d -> p t i d", p=LT))
            nc.sync.dma_start_transpose(out=st["kT"], in_=ext["k"][b])

        def stage_attn(b):
            st = state[b]
            q_sb, v_sb, kT_sb = st["q"], st["v"], st["kT"]
            psq = ps_small.tile([W, D], F32, tag="small")
            for i in range(NLT):
                nc.tensor.matmul(psq, lhsT=selT_sb[:, i, :], rhs=q_sb[:, i, :],
                                 start=(i == 0), stop=(i == NLT - 1))
            qb_sb = tiny.tile([W, D], F16, tag="qb")
            nc.scalar.copy(out=qb_sb, in_=psq)
            ptr5 = ps_small.tile([DT, NDT, 8], F16, tag="small")
            for i in range(NDT):
                nc.tensor.transpose(ptr5[:, i, :W], qb_sb[:, i * DT:(i + 1) * DT], id_sb)
            qbT_sb = tiny.tile([DT, NDT, W], F16, tag="qbT")
            nc.vector.tensor_copy(out=qbT_sb, in_=ptr5[:, :, :W])

            pqk = ps_small.tile([W, D], F32, tag="small")
            for i in range(NDT):
                nc.tensor.matmul(pqk, lhsT=qbT_sb[:, i, :], rhs=wqk_sb[:, i, :],
                                 start=(i == 0), stop=(i == NDT - 1))
            qk_sb = tiny.tile([W, D], F16, tag="qk")
            nc.scalar.copy(out=qk_sb, in_=pqk)
            ptrq = ps_small.tile([DT, NDT, 8], F16, tag="small")
            for i in range(NDT):
                nc.tensor.transpose(ptrq[:, i, :W], qk_sb[:, i * DT:(i + 1) * DT], id_sb)
            qkT_sb = tiny.tile([DT, NDT, W], F16, tag="qkT")
            nc.vector.tensor_copy(out=qkT_sb, in_=ptrq[:, :, :W])

            pS = ps_small.tile([W, L], F32, tag="small")
            for i in range(NDT):
                nc.tensor.matmul(pS, lhsT=qkT_sb[:, i, :], rhs=kT_sb[:, i, :L],
                                 start=(i == 0), stop=(i == NDT - 1))

            negmax = tiny.tile([W, 1], F32, tag="negmax")
            nc.vector.tensor_reduce(out=negmax, in_=pS, axis=mybir.AxisListType.X,
                                    op=mybir.AluOpType.max, negate=True)
            E_sb = tiny.tile([W, LP], F16, tag="E")
            sume = tiny.tile([W, 1], F32, tag="sume")
            nc.scalar.activation(out=E_sb[:, :L], in_=pS,
                                 func=mybir.ActivationFunctionType.Exp,
                                 bias=negmax, scale=1.0, accum_out=sume)
            nc.vector.memset(E_sb[:, L:], 0.0)
            rcp = tiny.tile([W, 1], F32, tag="rcp")
            nc.vector.reciprocal(out=rcp, in_=sume)
            A_sb = tiny.tile([W, LP], F16, tag="A")
            nc.vector.tensor_scalar_mul(out=A_sb, in0=E_sb, scalar1=rcp)

            ptrA = ps_small.tile([LT, NLT, 8], F16, tag="small")
            for i in range(NLT):
                nc.tensor.transpose(ptrA[:, i, :W], A_sb[:, i * LT:(i + 1) * LT], id_sb)
            AT_sb = tiny.tile([LT, NLT, W], F16, tag="AT")
            nc.vector.tensor_copy(out=AT_sb, in_=ptrA[:, :, :W])

            pt1 = ps_small.tile([W, D], F32, tag="small")
            for i in range(NLT):
                nc.tensor.matmul(pt1, lhsT=AT_sb[:, i, :], rhs=v_sb[:, i, :],
                                 start=(i == 0), stop=(i == NLT - 1))
            t1_sb = tiny.tile([W, D], F16, tag="t1")
            nc.scalar.copy(out=t1_sb, in_=pt1)
            ptrt = ps_small.tile([DT, NDT, 8], F16, tag="small")
            for i in range(NDT):
                nc.tensor.transpose(ptrt[:, i, :W], t1_sb[:, i * DT:(i + 1) * DT], id_sb)
            t1T_sb = tiny.tile([DT, NDT, W], F16, tag="t1T")
            nc.vector.tensor_copy(out=t1T_sb, in_=ptrt[:, :, :W])

            ppr = ps_small.tile([W, D], F32, tag="small")
            for i in range(NDT):
                nc.tensor.matmul(ppr, lhsT=t1T_sb[:, i, :], rhs=wvT_sb[:, i, :],
                                 start=(i == 0), stop=(i == NDT - 1))
            proto_sb = ptiny.tile([W, D], F16, tag="proto")
            nc.scalar.copy(out=proto_sb, in_=ppr)
            st["proto"] = proto_sb

        def stage_kp(b):
            st = state[b]
            kT_sb, proto_sb = st["kT"], st["proto"]
            x_sb = px.tile([LT, NLT, D], F16, tag="x")
            xT_sb = pxt.tile([DT, NDT, LP], F16, tag="xT")
            st1 = tiny.tile([LT, NLT, 6], F32, tag="st1")
            mv1 = tiny.tile([LT, NLT, 2], F32, tag="mv1")
            u1 = tiny.tile([LT, NLT], F32, tag="u1")
            rstd1 = tiny.tile([LT, NLT], F32, tag="rstd1")
            nb1 = tiny.tile([LT, NLT], F32, tag="nb1")
            pkps = {}
            for lt in range(NLT):
                pkp = ps_kp.tile([LT, D], F32, tag="kp", name=f"kp{b}_{lt}")
                pkps[lt] = pkp
                for dt in range(NDT):
                    nc.tensor.matmul(pkp, lhsT=kT_sb[:, dt, lt * LT:(lt + 1) * LT],
                                     rhs=wkT_sb[:, dt, :], start=(dt == 0), stop=False)
                nc.tensor.matmul(pkp, lhsT=bc5_sb[:, lt, :], rhs=proto_sb,
                                 start=False, stop=True)
                nc.vector.bn_stats(out=st1[:, lt, :], in_=pkp)
                nc.vector.bn_aggr(out=mv1[:, lt, :], in_=st1[:, lt, :])
                nc.scalar.activation(out=u1[:, lt:lt + 1], in_=mv1[:, lt, 1:2],
                                     func=mybir.ActivationFunctionType.Ln,
                                     bias=eps_sb, scale=1.0)
                nc.scalar.activation(out=rstd1[:, lt:lt + 1], in_=u1[:, lt:lt + 1],
                                     func=mybir.ActivationFunctionType.Exp,
                                     bias=0.0, scale=-0.5)
                nc.vector.scalar_tensor_tensor(out=nb1[:, lt:lt + 1],
                                               in0=mv1[:, lt, 0:1], scalar=-1.0,
                                               in1=rstd1[:, lt:lt + 1],
                                               op0=mybir.AluOpType.mult,
                                               op1=mybir.AluOpType.mult)
                nc.scalar.activation(out=x_sb[:, lt, :], in_=pkps[lt],
                                     func=mybir.ActivationFunctionType.Identity,
                                     bias=nb1[:, lt:lt + 1],
                                     scale=rstd1[:, lt:lt + 1])
                if apply_gb:
                    nc.vector.tensor_mul(out=x_sb[:, lt, :], in0=x_sb[:, lt, :],
                                         in1=gb_sb["g1"])
                    nc.vector.tensor_add(out=x_sb[:, lt, :], in0=x_sb[:, lt, :],
                                         in1=gb_sb["b1"])
                nc.sync.dma_start_transpose(out=xT_sb[:, :, lt * LT:(lt + 1) * LT],
                                            in_=x_sb[:, lt, :])
            st["x"] = x_sb
            st["xT"] = xT_sb

        def stage_fc(b):
            st = state[b]
            x_sb, xT_sb = st["x"], st["xT"]
            t_sb = pt.tile([LT, NLT, D], F32, tag="t")
            r_sb = pr.tile([LT, NLT, D], F32, tag="r")
            o_sb = po.tile([LT, NLT, D], F16, tag="o")
            st2 = tiny.tile([LT, NLT, 6], F32, tag="st2")
            mv2 = tiny.tile([LT, NLT, 2], F32, tag="mv2")
            u2 = tiny.tile([LT, NLT], F32, tag="u2")
            rstd2 = tiny.tile([LT, NLT], F32, tag="rstd2")
            for lt in range(NLT):
                py = ps_fc.tile([LT, D], F32, tag="fc")
                for et in range(NDT):
                    nc.tensor.matmul(py, lhsT=xT_sb[:, et, lt * LT:(lt + 1) * LT],
                                     rhs=wfcT_sb[:, et, :],
                                     start=(et == 0), stop=(et == NDT - 1))
                a_lt = t_sb[:, lt, :]
                # leaky(z) = z + (1-LEAK)*relu(-z)
                nc.scalar.activation(out=a_lt, in_=py,
                                     func=mybir.ActivationFunctionType.Relu,
                                     bias=0.0, scale=-1.0)
                nc.vector.scalar_tensor_tensor(out=a_lt, in0=a_lt, scalar=(1.0 - LEAK),
                                               in1=py, op0=mybir.AluOpType.mult,
                                               op1=mybir.AluOpType.add)
                nc.gpsimd.tensor_add(out=r_sb[:, lt, :], in0=a_lt, in1=x_sb[:, lt, :])
                nc.vector.bn_stats(out=st2[:, lt, :], in_=r_sb[:, lt, :])
                nc.vector.bn_aggr(out=mv2[:, lt, :], in_=st2[:, lt, :])
            nc.scalar.activation(out=u2, in_=mv2[:, :, 1],
                                 func=mybir.ActivationFunctionType.Ln,
                                 bias=eps_sb, scale=1.0)
            nc.scalar.activation(out=rstd2, in_=u2,
                                 func=mybir.ActivationFunctionType.Exp,
                                 bias=0.0, scale=-0.5)
            for lt in range(NLT):
                nc.vector.tensor_scalar(out=o_sb[:, lt, :], in0=r_sb[:, lt, :],
                                        scalar1=mv2[:, lt, 0:1],
                                        scalar2=rstd2[:, lt:lt + 1],
                                        op0=mybir.AluOpType.subtract,
                                        op1=mybir.AluOpType.mult)
                if apply_gb:
                    nc.vector.tensor_mul(out=o_sb[:, lt, :], in0=o_sb[:, lt, :],
                                         in1=gb_sb["g2"])
                    nc.vector.tensor_add(out=o_sb[:, lt, :], in0=o_sb[:, lt, :],
                                         in1=gb_sb["b2"])
            nc.sync.dma_start(out=ext["out"][b].rearrange("(i p) d -> p i d", p=LT),
                              in_=o_sb)
            del state[b]

        # pipelined emission
        stage_load(0)
        stage_load(1)
        stage_attn(0)
        for s in range(BPC):
            if s + 2 < BPC:
                stage_load(s + 2)
            if s + 1 < BPC:
                stage_attn(s + 1)
            stage_kp(s)
            if s >= 1:
                stage_fc(s - 1)
        stage_fc(BPC - 1)


_PROGRAM_CACHE = {}


def _build(apply_gb):
    key = bool(apply_gb)
    if key in _PROGRAM_CACHE:
        return _PROGRAM_CACHE[key]
    nc = bacc.Bacc("TRN2", target_bir_lowering=False, debug=False,
                   num_devices=N_CORES)
    ext = {}
    ext["qv"] = nc.declare_dram_parameter("qv", [BPC, 2, LP, D], F16, isOutput=False)
    ext["k"] = nc.declare_dram_parameter("k", [BPC, LP, D], F16, isOutput=False)
    for name in ("wkT", "wvT", "wfcT", "wqk"):
        ext[name] = nc.declare_dram_parameter(name, [D, D], F16, isOutput=False)
    ext["ident"] = nc.declare_dram_parameter("ident", [W, W], F16, isOutput=False)
    ext["selT"] = nc.declare_dram_parameter("selT", [LP, W], F16, isOutput=False)
    ext["bc5"] = nc.declare_dram_parameter("bc5", [W, LP], F16, isOutput=False)
    if apply_gb:
        for name in ("g1", "b1", "g2", "b2"):
            ext[name] = nc.declare_dram_parameter(name, [D], F32, isOutput=False)
    ext["out"] = nc.declare_dram_parameter("out", [BPC, LP, D], F16, isOutput=True)

    with tile.TileContext(nc) as tc:
        _emit(nc, tc, ext, apply_gb)
    nc.compile()
    _PROGRAM_CACHE[key] = (nc, apply_gb)
    return _PROGRAM_CACHE[key]


def kernel(q, k, v, Wq, Wk, Wv, Wfc, g1, b1, g2, b2, _trace=False):
    q = np.asarray(q, dtype=np.float32)
    k = np.asarray(k, dtype=np.float32)
    v = np.asarray(v, dtype=np.float32)
    Wq = np.asarray(Wq, dtype=np.float32)
    Wk = np.asarray(Wk, dtype=np.float32)
    Wv = np.asarray(Wv, dtype=np.float32)
    Wfc = np.asarray(Wfc, dtype=np.float32)
    g1 = np.asarray(g1, dtype=np.float32)
    b1 = np.asarray(b1, dtype=np.float32)
    g2 = np.asarray(g2, dtype=np.float32)
    b2 = np.asarray(b2, dtype=np.float32)

    apply_gb = not (np.all(g1 == 1) and np.all(b1 == 0)
                    and np.all(g2 == 1) and np.all(b2 == 0))

    def pad16(x):
        out = np.zeros((BPC * N_CORES, LP, D), dtype=np.float16)
        out[:, :L, :] = x.astype(np.float16)
        return out

    q16, k16, v16 = pad16(q), pad16(k), pad16(v)
    qv16 = np.ascontiguousarray(np.stack([q16, v16], axis=1))
    wkT = np.ascontiguousarray(Wk.T).astype(np.float16)
    wvT = np.ascontiguousarray(Wv.T).astype(np.float16)
    wfcT = np.ascontiguousarray(Wfc.T).astype(np.float16)
    wqk = ((Wq.T.astype(np.float64) @ Wk.astype(np.float64)) / TEMP).astype(np.float16)
    ident = np.eye(W, dtype=np.float16)
    sel = np.zeros((LP, W), dtype=np.float16)
    sel[np.arange(L), np.arange(L) % W] = np.float16(W / L)
    bc5 = np.zeros((W, LP), dtype=np.float16)
    bc5[np.arange(L) % W, np.arange(L)] = 1.0

    nc, _ = _build(apply_gb)

    in_maps = []
    for c in range(N_CORES):
        m = {
            "qv": qv16[c * BPC:(c + 1) * BPC],
            "k": k16[c * BPC:(c + 1) * BPC],
            "wkT": wkT, "wvT": wvT, "wfcT": wfcT, "wqk": wqk,
            "ident": ident, "selT": sel, "bc5": bc5,
        }
        if apply_gb:
            m.update({"g1": g1, "b1": b1, "g2": g2, "b2": b2})
        in_maps.append(m)

    res = run_bass_kernel_spmd(nc, in_maps, core_ids=list(range(N_CORES)),
                               trace=_trace)
    out = np.concatenate([res.results[c]["out"] for c in range(N_CORES)], axis=0)[:, :L, :].astype(np.float32)
    if _trace:
        kernel._last_results = res
    return out


# revision 28
# speedup vs baseline: 170.9844x; 170.9844x over previous
"""Trainium2 Bass kernel for nn_MultiHeadAttention_18700287607660.

Math (B=128, L=500, D=512, NWAY=5, n_head=1):
  qp = q@Wq.T ; kp = k@Wk.T ; vp = v@Wv.T
  attn_avg = softmax(mean_over_groups(qp @ kp.T / temp))     # [B, 5, L]
  proto = attn_avg @ vp                                      # [B, 5, D]
  out1 = LN1(broadcast(proto) + kp)
  out  = LN2(leaky_relu(out1@Wfc.T, 0.1) + out1)

Key restructurings (exact up to fp reassociation):
  * mean happens BEFORE softmax, so the [500,500] attention matrix is never
    formed:  S = (Sel @ q) @ (Wq.T @ Wk / temp) @ k.T   with Sel the [5,500]
    group-mean selector. Wqk = Wq.T@Wk/temp folded on the host.
  * proto = (A @ v) @ Wv.T  — the V projection is never materialized.
  * broadcast(proto) is a K=5 matmul accumulated straight into kp's PSUM.

Implementation choices:
  * matmul operands in fp16 — fp32 moving operands stream at half rate on the
    PE; fp16 keeps ~1e-3 accuracy with fp32 PSUM accumulation.
  * seq dim host-padded 500->512 (zero rows) so the DMA xbar transpose engine
    (2-byte dtypes, rows%16==0) produces k^T and x^T — no PE transposes or
    PSUM->SBUF copybacks for the two big transposed tensors.
  * LayerNorm rstd via exp(-0.5*ln(var+eps)): keeps every ACT function in the
    one "natural_log_exp_and_others" table set (one table load total).

Sharding: pure data parallel, 16 batches per core across 8 cores.
"""
import os
import sys

for _p in ("/opt/trn_rl_repo", "/root/.axon_site/_ro/trn_rl_repo"):
    if os.path.isdir(_p) and _p not in sys.path:
        sys.path.insert(0, _p)

import numpy as np

import concourse.bacc as bacc
import concourse.bass as bass
import concourse.tile as tile
from concourse import mybir
from concourse.bass_utils import run_bass_kernel_spmd

F16 = mybir.dt.float16
F32 = mybir.dt.float32
N_CORES = 8
B = 128
BPC = B // N_CORES   # 16 batches per core
L = 500              # true seq len
LP = 512             # padded seq len (DMA xbar transpose needs rows%16==0)
LT = 128             # l-tile
NLT = LP // LT       # 4
LTAIL = L - 3 * LT   # 116 valid rows in the last l-tile
D = 512
DT = 128
NDT = D // DT        # 4
W = 5                # NWAY shot groups
TEMP = float(np.sqrt(float(D)))
EPS = 1e-6
LEAK = 0.1

# All ACT functions used here (Exp, Ln, Relu, Copy, Identity) live in the
# "natural_log_exp_and_others" table set, but bacc's per-activation greedy
# set chooser still flips between sets (hundreds of ~2.7us ACT_TABLE_LOADs).
# Empty out every other set (keeping positions, since act_func_set_id is the
# positional index into act_info.json) so exactly one set is ever loaded.
_orig_get_activation_tables = bacc.get_activation_tables


def _pinned_activation_tables(module_arch):
    tables = _orig_get_activation_tables(module_arch)
    if "natural_log_exp_and_others" in tables:
        return {
            name: (fns if name == "natural_log_exp_and_others" else set())
            for name, fns in tables.items()
        }
    return tables


bacc.get_activation_tables = _pinned_activation_tables


def _emit(nc, tc, ext, apply_gb):
    """Software-pipelined emission: per iteration s we emit
    load(s+2) | attn(s+1) | kp+LN1+xT(s) | fc+LN2+store(s-1)
    so every stage's inputs were produced in a previous iteration and the
    per-engine streams always have ready work from an adjacent batch.
    """
    import contextlib
    ctx = contextlib.ExitStack()
    with ctx:
        const = ctx.enter_context(tc.tile_pool(name="const", bufs=1))
        pin = ctx.enter_context(tc.tile_pool(name="pin", bufs=3))
        pkt = ctx.enter_context(tc.tile_pool(name="pkt", bufs=4))
        px = ctx.enter_context(tc.tile_pool(name="px", bufs=3))
        pxt = ctx.enter_context(tc.tile_pool(name="pxt", bufs=3))
        pt = ctx.enter_context(tc.tile_pool(name="pt", bufs=3))
        pr = ctx.enter_context(tc.tile_pool(name="pr", bufs=3))
        po = ctx.enter_context(tc.tile_pool(name="po", bufs=3))
        tiny = ctx.enter_context(tc.tile_pool(name="tiny", bufs=3))
        ptiny = ctx.enter_context(tc.tile_pool(name="ptiny", bufs=3))
        ps_small = ctx.enter_context(tc.tile_pool(name="ps_small", bufs=2, space="PSUM"))
        ps_kp = ctx.enter_context(tc.tile_pool(name="ps_kp", bufs=4, space="PSUM"))
        ps_fc = ctx.enter_context(tc.tile_pool(name="ps_fc", bufs=2, space="PSUM"))

        # ---- constants ----
        wkT_sb = const.tile([DT, NDT, D], F16)
        wvT_sb = const.tile([DT, NDT, D], F16)
        wfcT_sb = const.tile([DT, NDT, D], F16)
        wqk_sb = const.tile([DT, NDT, D], F16)
        for w_sb, name in ((wkT_sb, "wkT"), (wvT_sb, "wvT"),
                           (wfcT_sb, "wfcT"), (wqk_sb, "wqk")):
            nc.sync.dma_start(out=w_sb, in_=ext[name].rearrange("(i p) e -> p i e", p=DT))
        selT_sb = const.tile([LT, NLT, W], F16)
        nc.sync.dma_start(out=selT_sb, in_=ext["selT"].rearrange("(i p) w -> p i w", p=LT))
        bc5_sb = const.tile([DT, NLT, LT], F16)
        nc.sync.dma_start(out=bc5_sb, in_=ext["bc5"].rearrange("w (i p) -> w i p", p=LT))
        id_sb = const.tile([W, W], F16)
        nc.sync.dma_start(out=id_sb, in_=ext["ident"][:])
        eps_sb = const.tile([DT, 1], F32)
        nc.vector.memset(eps_sb, EPS)
        gb_sb = {}
        if apply_gb:
            for name in ("g1", "b1", "g2", "b2"):
                t = const.tile([LT, D], F32)
                src = ext[name]
                bcast = bass.AP(tensor=src.tensor, offset=src.offset,
                                ap=[[0, LT]] + list(src.ap))
                nc.sync.dma_start(out=t, in_=bcast)
                gb_sb[name] = t

        state = {}

        def stage_load(b):
            st = state.setdefault(b, {})
            qv = pin.tile([LT, 2, NLT, D], F16, tag="qv", name=f"qv{b}")
            st["q"] = qv[:, 0, :, :]
            st["v"] = qv[:, 1, :, :]
            st["kT"] = pkt.tile([DT, NDT, LP], F16, tag="kT", name=f"kT{b}")
            nc.sync.dma_start(out=qv, in_=ext["qv"][b].rearrange("t (i p) d -> p t i d", p=LT))
            nc.sync.dma_start_transpose(out=st["kT"], in_=ext["k"][b])

        def stage_attn(b):
            st = state[b]
            q_sb, v_sb, kT_sb = st["q"], st["v"], st["kT"]
            psq = ps_small.tile([W, D], F32, tag="small")
            for i in range(NLT):
                nc.tensor.matmul(psq, lhsT=selT_sb[:, i, :], rhs=q_sb[:, i, :],
                                 start=(i == 0), stop=(i == NLT - 1))
            qb_sb = tiny.tile([W, D], F16, tag="qb")
            nc.scalar.copy(out=qb_sb, in_=psq)
            ptr5 = ps_small.tile([DT, NDT, 8], F16, tag="small")
            for i in range(NDT):
                nc.tensor.transpose(ptr5[:, i, :W], qb_sb[:, i * DT:(i + 1) * DT], id_sb)
            qbT_sb = tiny.tile([DT, NDT, W], F16, tag="qbT")
            nc.vector.tensor_copy(out=qbT_sb, in_=ptr5[:, :, :W])

            pqk = ps_small.tile([W, D], F32, tag="small")
            for i in range(NDT):
                nc.tensor.matmul(pqk, lhsT=qbT_sb[:, i, :], rhs=wqk_sb[:, i, :],
                                 start=(i == 0), stop=(i == NDT - 1))
            qk_sb = tiny.tile([W, D], F16, tag="qk")
            nc.scalar.copy(out=qk_sb, in_=pqk)
            ptrq = ps_small.tile([DT, NDT, 8], F16, tag="small")
            for i in range(NDT):
                nc.tensor.transpose(ptrq[:, i, :W], qk_sb[:, i * DT:(i + 1) * DT], id_sb)
            qkT_sb = tiny.tile([DT, NDT, W], F16, tag="qkT")
            nc.vector.tensor_copy(out=qkT_sb, in_=ptrq[:, :, :W])

            pS = ps_small.tile([W, L], F32, tag="small")
            for i in range(NDT):
                nc.tensor.matmul(pS, lhsT=qkT_sb[:, i, :], rhs=kT_sb[:, i, :L],
                                 start=(i == 0), stop=(i == NDT - 1))

            negmax = tiny.tile([W, 1], F32, tag="negmax")
            nc.vector.tensor_reduce(out=negmax, in_=pS, axis=mybir.AxisListType.X,
                                    op=mybir.AluOpType.max, negate=True)
            E_sb = tiny.tile([W, LP], F16, tag="E")
            sume = tiny.tile([W, 1], F32, tag="sume")
            nc.scalar.activation(out=E_sb[:, :L], in_=pS,
                                 func=mybir.ActivationFunctionType.Exp,
                                 bias=negmax, scale=1.0, accum_out=sume)
            nc.vector.memset(E_sb[:, L:], 0.0)
            rcp = tiny.tile([W, 1], F32, tag="rcp")
            nc.vector.reciprocal(out=rcp, in_=sume)
            A_sb = tiny.tile([W, LP], F16, tag="A")
            nc.vector.tensor_scalar_mul(out=A_sb, in0=E_sb, scalar1=rcp)

            ptrA = ps_small.tile([LT, NLT, 8], F16, tag="small")
            for i in range(NLT):
                nc.tensor.transpose(ptrA[:, i, :W], A_sb[:, i * LT:(i + 1) * LT], id_sb)
            AT_sb = tiny.tile([LT, NLT, W], F16, tag="AT")
            nc.vector.tensor_copy(out=AT_sb, in_=ptrA[:, :, :W])

            pt1 = ps_small.tile([W, D], F32, tag="small")
            for i in range(NLT):
                nc.tensor.matmul(pt1, lhsT=AT_sb[:, i, :], rhs=v_sb[:, i, :],
                                 start=(i == 0), stop=(i == NLT - 1))
            t1_sb = tiny.tile([W, D], F16, tag="t1")
            nc.scalar.copy(out=t1_sb, in_=pt1)
            ptrt = ps_small.tile([DT, NDT, 8], F16, tag="small")
            for i in range(NDT):
                nc.tensor.transpose(ptrt[:, i, :W], t1_sb[:, i * DT:(i + 1) * DT], id_sb)
            t1T_sb = tiny.tile([DT, NDT, DT], F16, tag="t1T")
            nc.vector.memset(t1T_sb, 0.0)
            # replicate t1T's 5 columns into 4x 32-col groups (write AP [32x4, 1x5])
            rd = ptrt[:, :, :W]
            rep_in = bass.AP(tensor=rd.tensor, offset=rd.offset,
                             ap=[list(rd.ap[0]), list(rd.ap[1]), [0, 4], list(rd.ap[2])])
            wr = t1T_sb
            rep_out = bass.AP(tensor=wr.tensor, offset=wr.offset,
                              ap=[list(wr.ap[0]), list(wr.ap[1]), [32, 4], [1, W]])
            nc.vector.tensor_copy(out=rep_out, in_=rep_in)

            ppr = ps_small.tile([DT, D], F32, tag="small")
            for i in range(NDT):
                nc.tensor.matmul(ppr, lhsT=t1T_sb[:, i, :], rhs=wvT_sb[:, i, :],
                                 start=(i == 0), stop=(i == NDT - 1))
            proto_sb = ptiny.tile([DT, D], F16, tag="proto")
            nc.scalar.copy(out=proto_sb, in_=ppr)
            st["proto"] = proto_sb

        def stage_kp(b):
            st = state[b]
            kT_sb, proto_sb = st["kT"], st["proto"]
            x_sb = px.tile([LT, NLT, D], F16, tag="x")
            xT_sb = pxt.tile([DT, NDT, LP], F16, tag="xT")
            st1 = tiny.tile([LT, NLT, 6], F32, tag="st1")
            mv1 = tiny.tile([LT, NLT, 2], F32, tag="mv1")
            u1 = tiny.tile([LT, NLT], F32, tag="u1")
            rstd1 = tiny.tile([LT, NLT], F32, tag="rstd1")
            nb1 = tiny.tile([LT, NLT], F32, tag="nb1")
            pkps = {}
            for lt in range(NLT):
                pkp = ps_kp.tile([LT, D], F32, tag="kp", name=f"kp{b}_{lt}")
                pkps[lt] = pkp
                for dt in range(NDT):
                    nc.tensor.matmul(pkp, lhsT=kT_sb[:, dt, lt * LT:(lt + 1) * LT],
                                     rhs=wkT_sb[:, dt, :], start=(dt == 0), stop=False)
            for lt in range(NLT):
                nc.tensor.matmul(pkps[lt], lhsT=bc5_sb[32 * lt:32 * lt + W, lt, :],
                                 rhs=proto_sb[32 * lt:32 * lt + W, :],
                                 start=False, stop=True, tile_position=(32 * lt, 0))
            for lt in range(NLT):
                pkp = pkps[lt]
                nc.vector.bn_stats(out=st1[:, lt, :], in_=pkp)
                nc.vector.bn_aggr(out=mv1[:, lt, :], in_=st1[:, lt, :])
                nc.scalar.activation(out=u1[:, lt:lt + 1], in_=mv1[:, lt, 1:2],
                                     func=mybir.ActivationFunctionType.Ln,
                                     bias=eps_sb, scale=1.0)
                nc.scalar.activation(out=rstd1[:, lt:lt + 1], in_=u1[:, lt:lt + 1],
                                     func=mybir.ActivationFunctionType.Exp,
                                     bias=0.0, scale=-0.5)
                nc.vector.scalar_tensor_tensor(out=nb1[:, lt:lt + 1],
                                               in0=mv1[:, lt, 0:1], scalar=-1.0,
                                               in1=rstd1[:, lt:lt + 1],
                                               op0=mybir.AluOpType.mult,
                                               op1=mybir.AluOpType.mult)
                nc.scalar.activation(out=x_sb[:, lt, :], in_=pkps[lt],
                                     func=mybir.ActivationFunctionType.Identity,
                                     bias=nb1[:, lt:lt + 1],
                                     scale=rstd1[:, lt:lt + 1])
                if apply_gb:
                    nc.vector.tensor_mul(out=x_sb[:, lt, :], in0=x_sb[:, lt, :],
                                         in1=gb_sb["g1"])
                    nc.vector.tensor_add(out=x_sb[:, lt, :], in0=x_sb[:, lt, :],
                                         in1=gb_sb["b1"])
                nc.sync.dma_start_transpose(out=xT_sb[:, :, lt * LT:(lt + 1) * LT],
                                            in_=x_sb[:, lt, :])
            st["x"] = x_sb
            st["xT"] = xT_sb

        def stage_fc(b):
            st = state[b]
            x_sb, xT_sb = st["x"], st["xT"]
            t_sb = pt.tile([LT, NLT, D], F32, tag="t")
            r_sb = pr.tile([LT, NLT, D], F32, tag="r")
            o_sb = po.tile([LT, NLT, D], F16, tag="o")
            st2 = tiny.tile([LT, NLT, 6], F32, tag="st2")
            mv2 = tiny.tile([LT, NLT, 2], F32, tag="mv2")
            u2 = tiny.tile([LT, NLT], F32, tag="u2")
            rstd2 = tiny.tile([LT, NLT], F32, tag="rstd2")
            for lt in range(NLT):
                py = ps_fc.tile([LT, D], F32, tag="fc")
                for et in range(NDT):
                    nc.tensor.matmul(py, lhsT=xT_sb[:, et, lt * LT:(lt + 1) * LT],
                                     rhs=wfcT_sb[:, et, :],
                                     start=(et == 0), stop=(et == NDT - 1))
                a_lt = t_sb[:, lt, :]
                # leaky(z) = z + (1-LEAK)*relu(-z)
                nc.scalar.activation(out=a_lt, in_=py,
                                     func=mybir.ActivationFunctionType.Relu,
                                     bias=0.0, scale=-1.0)
                nc.vector.scalar_tensor_tensor(out=a_lt, in0=a_lt, scalar=(1.0 - LEAK),
                                               in1=py, op0=mybir.AluOpType.mult,
                                               op1=mybir.AluOpType.add)
                nc.gpsimd.tensor_add(out=r_sb[:, lt, :], in0=a_lt, in1=x_sb[:, lt, :])
                nc.vector.bn_stats(out=st2[:, lt, :], in_=r_sb[:, lt, :])
                nc.vector.bn_aggr(out=mv2[:, lt, :], in_=st2[:, lt, :])
            nc.scalar.activation(out=u2, in_=mv2[:, :, 1],
                                 func=mybir.ActivationFunctionType.Ln,
                                 bias=eps_sb, scale=1.0)
            nc.scalar.activation(out=rstd2, in_=u2,
                                 func=mybir.ActivationFunctionType.Exp,
                                 bias=0.0, scale=-0.5)
            for lt in range(NLT):
                nc.vector.tensor_scalar(out=o_sb[:, lt, :], in0=r_sb[:, lt, :],
                                        scalar1=mv2[:, lt, 0:1],
                                        scalar2=rstd2[:, lt:lt + 1],
                                        op0=mybir.AluOpType.subtract,
                                        op1=mybir.AluOpType.mult)
                if apply_gb:
                    nc.vector.tensor_mul(out=o_sb[:, lt, :], in0=o_sb[:, lt, :],
                                         in1=gb_sb["g2"])
                    nc.vector.tensor_add(out=o_sb[:, lt, :], in0=o_sb[:, lt, :],
                                         in1=gb_sb["b2"])
            nc.sync.dma_start(out=ext["out"][b].rearrange("(i p) d -> p i d", p=LT),
                              in_=o_sb)
            del state[b]


        def stage_kp_fc_interleaved(bk, bf):
            stage_kp(bk)
            stage_fc(bf)
        # pipelined emission
        stage_load(0)
        stage_load(1)
        stage_attn(0)
        for s in range(BPC):
            if s + 2 < BPC:
                stage_load(s + 2)
            if s + 1 < BPC:
                stage_attn(s + 1)
            if s >= 1:
                stage_kp_fc_interleaved(s, s - 1)
            else:
                stage_kp(s)
        stage_fc(BPC - 1)


_PROGRAM_CACHE = {}


def _build(apply_gb):
    key = bool(apply_gb)
    if key in _PROGRAM_CACHE:
        return _PROGRAM_CACHE[key]
    nc = bacc.Bacc("TRN2", target_bir_lowering=False, debug=False,
                   num_devices=N_CORES)
    ext = {}
    ext["qv"] = nc.declare_dram_parameter("qv", [BPC, 2, LP, D], F16, isOutput=False)
    ext["k"] = nc.declare_dram_parameter("k", [BPC, LP, D], F16, isOutput=False)
    for name in ("wkT", "wvT", "wfcT", "wqk"):
        ext[name] = nc.declare_dram_parameter(name, [D, D], F16, isOutput=False)
    ext["ident"] = nc.declare_dram_parameter("ident", [W, W], F16, isOutput=False)
    ext["selT"] = nc.declare_dram_parameter("selT", [LP, W], F16, isOutput=False)
    ext["bc5"] = nc.declare_dram_parameter("bc5", [DT, LP], F16, isOutput=False)
    if apply_gb:
        for name in ("g1", "b1", "g2", "b2"):
            ext[name] = nc.declare_dram_parameter(name, [D], F32, isOutput=False)
    ext["out"] = nc.declare_dram_parameter("out", [BPC, LP, D], F16, isOutput=True)

    with tile.TileContext(nc) as tc:
        _emit(nc, tc, ext, apply_gb)
    nc.compile()
    _PROGRAM_CACHE[key] = (nc, apply_gb)
    return _PROGRAM_CACHE[key]


def kernel(q, k, v, Wq, Wk, Wv, Wfc, g1, b1, g2, b2, _trace=False):
    q = np.asarray(q, dtype=np.float32)
    k = np.asarray(k, dtype=np.float32)
    v = np.asarray(v, dtype=np.float32)
    Wq = np.asarray(Wq, dtype=np.float32)
    Wk = np.asarray(Wk, dtype=np.float32)
    Wv = np.asarray(Wv, dtype=np.float32)
    Wfc = np.asarray(Wfc, dtype=np.float32)
    g1 = np.asarray(g1, dtype=np.float32)
    b1 = np.asarray(b1, dtype=np.float32)
    g2 = np.asarray(g2, dtype=np.float32)
    b2 = np.asarray(b2, dtype=np.float32)

    apply_gb = not (np.all(g1 == 1) and np.all(b1 == 0)
                    and np.all(g2 == 1) and np.all(b2 == 0))

    def pad16(x):
        out = np.zeros((BPC * N_CORES, LP, D), dtype=np.float16)
        out[:, :L, :] = x.astype(np.float16)
        return out

    q16, k16, v16 = pad16(q), pad16(k), pad16(v)
    qv16 = np.ascontiguousarray(np.stack([q16, v16], axis=1))
    wkT = np.ascontiguousarray(Wk.T).astype(np.float16)
    wvT = np.ascontiguousarray(Wv.T).astype(np.float16)
    wfcT = np.ascontiguousarray(Wfc.T).astype(np.float16)
    wqk = ((Wq.T.astype(np.float64) @ Wk.astype(np.float64)) / TEMP).astype(np.float16)
    ident = np.eye(W, dtype=np.float16)
    sel = np.zeros((LP, W), dtype=np.float16)
    sel[np.arange(L), np.arange(L) % W] = np.float16(W / L)
    bc5 = np.zeros((DT, LP), dtype=np.float16)
    for _l in range(L):
        bc5[32 * (_l // LT) + _l % W, _l] = 1.0

    nc, _ = _build(apply_gb)

    in_maps = []
    for c in range(N_CORES):
        m = {
            "qv": qv16[c * BPC:(c + 1) * BPC],
            "k": k16[c * BPC:(c + 1) * BPC],
            "wkT": wkT, "wvT": wvT, "wfcT": wfcT, "wqk": wqk,
            "ident": ident, "selT": sel, "bc5": bc5,
        }
        if apply_gb:
            m.update({"g1": g1, "b1": b1, "g2": g2, "b2": b2})
        in_maps.append(m)

    res = run_bass_kernel_spmd(nc, in_maps, core_ids=list(range(N_CORES)),
                               trace=_trace)
    out = np.concatenate([res.results[c]["out"] for c in range(N_CORES)], axis=0)[:, :L, :].astype(np.float32)
    if _trace:
        kernel._last_results = res
    return out
